# revision 10
# baseline (speedup 1.0000x reference)
"""LocalPoolPointnet on 8 Trainium2 cores (v3).

Points sorted by (batch, sparse-bin) on the host; each core owns a contiguous
bin range of one batch plus all points in it.  scatter_mean / gather are
core-local one-hot matmuls in fp32.  v3 structural changes vs the original:

- one dynamic 512-bin window per PAIR of 512-pt chunks (max span 455 < 512):
  halves window adds, snaps, wstage copies and mean transposes.
- raw bin sums on device; per-bin 1/count folded into the mean-window
  transpose evac (recipW, host-precomputed per pair-block); the final output
  is divided by counts on the host.
- gather one-hot is a plain is_equal (no per-point recip multiply, no rp
  broadcast DMA); scatter one-hot built by gpsimd local_scatter (or DVE
  is_equal fallback), freeing the vector engine.
- gather matmuls and resblock evacs run on 1024-pt tiles; independent psum
  banks are interleaved (scatter uses two alternating accumulators) so fp32
  matmul passes overlap.
"""

import sys
import numpy as np

# ---------------------------------------------------------------- constants
B = 2
NP_ = 100_000
HID = 128
D2 = 256
NBLK = 5
RES = 64
R = 20_005          # max_coord_num in the reference
BIG = RES ** 3 + 1
NCORES = 8
CORES_PER_BATCH = NCORES // B

NPTS = 25_600       # padded points per core  (= 50 * 512)
NCHUNK = NPTS // 512            # 50
NPAIR = NCHUNK // 2             # 25
NSTRIP = NPTS // 128            # 200
WIN = 512                       # bin window per 1024-pt pair
NBINS = 6_016                   # padded bins per core (= 47 * 128)
NBIAS = 13                      # b_pos(2) b0(5) b1(5) b_c(1)
FW = NPTS * 4 // 128            # 784

USE_LOCAL_SCATTER = False       # local_scatter needs 2-byte out: unusable

F32 = np.float32
F16 = np.float16


# ================================================================ host prep
def point_meta(p, sparse_coords, res):
    """Integer routing metadata, bit-identical to the reference's indexing."""
    p = np.asarray(p, F32)
    sc = np.asarray(sparse_coords)
    coord = np.clip(p + F32(0.5), F32(1e-6), F32(1.0 - 1e-6)) * F32(res)
    cl = coord.astype(np.int32)
    lin = (cl[..., 0] * res + cl[..., 1]) * res + cl[..., 2]      # [B, NP]

    slin = (sc[:, 1] * res + sc[:, 2]) * res + sc[:, 3]
    index = np.empty((B, NP_), np.int64)
    for b in range(B):
        coords_b = np.sort(np.where(sc[:, 0] == b, slin, BIG))
        index[b] = np.searchsorted(coords_b, lin[b], side="left")
    counts = np.bincount(sc[:, 0], minlength=B)
    return index, counts


def shard(p, index):
    """Split each batch's points into CORES_PER_BATCH contiguous-bin shards."""
    shards = []
    for b in range(B):
        idx = index[b]
        order = np.argsort(idx, kind="stable")
        sidx = idx[order]
        binc = np.bincount(idx, minlength=R)
        csum = np.cumsum(binc)
        prev_hi = 0
        for c in range(CORES_PER_BATCH):
            if c < CORES_PER_BATCH - 1:
                target = (c + 1) * NP_ // CORES_PER_BATCH
                hi = int(np.searchsorted(csum, target))
                if hi > 0 and target - csum[hi - 1] < csum[hi] - target:
                    hi -= 1
                hi += 1          # shard owns bins [lo, hi)
            else:
                hi = R
            lo = prev_hi
            prev_hi = hi
            sel = slice(int(np.searchsorted(sidx, lo)), int(np.searchsorted(sidx, hi)))
            pts = p[b][order[sel]]                     # [n, 3] sorted by bin
            rel = (sidx[sel] - lo).astype(np.int64)    # sorted rel bins
            assert pts.shape[0] <= NPTS, f"core shard too big: {pts.shape[0]}"
            nb = hi - lo
            assert nb <= NBINS, f"bin shard too big: {nb}"
            shards.append(dict(batch=b, lo=lo, hi=hi, pts=pts, rel=rel, nb=nb))
    return shards


def core_inputs(sh):
    """Per-core padded arrays for the device kernel."""
    n = sh["pts"].shape[0]
    pts = np.full((NPTS, 3), 0.25, F32)
    pts[:n] = sh["pts"]
    rel = sh["rel"]

    lb = np.full(NPTS, -1.0, F32)       # bin - pair window base (-1 dummies)
    wbase = np.zeros(NPAIR, np.int32)   # window base per 1024-pt pair
    for P in range(NPAIR):
        s, e = P * 1024, min((P + 1) * 1024, n)
        if s >= n:
            break
        base = min(int(rel[s]), NBINS - WIN)
        span = int(rel[e - 1]) - base + 1
        assert span <= WIN, f"window overflow: span={span}"
        wbase[P] = base
        lb[s:e] = (rel[s:e] - base).astype(F32)

    cnt = np.bincount(rel, minlength=NBINS).astype(F32)
    recip = F32(1.0) / np.maximum(cnt, F32(1.0))
    recip_pad = np.concatenate([recip, np.ones(WIN, F32)])
    recipW = np.zeros((128, 4 * NPAIR), F32)
    for P in range(NPAIR):
        for bl in range(4):
            recipW[:, 4 * P + bl] = recip_pad[wbase[P] + 128 * bl:
                                              wbase[P] + 128 * bl + 128]

    pts4 = np.zeros((4, NPTS), F32)
    pts4[:3] = pts.T
    pts_flat = np.ascontiguousarray(pts4).reshape(128, FW)
    lbT = np.ascontiguousarray(lb.reshape(NSTRIP, 128).T)          # [128, NSTRIP]
    lbTi = lbT.astype(np.int32)
    gbr = lb.reshape(NPAIR, 1024).astype(F16)
    wb = np.zeros((1, 32), np.int32)
    wb[0, :NPAIR] = wbase
    return dict(pts_flat=pts_flat, lbT=lbT, lbTi=lbTi, gbr=gbr, wbase=wb,
                recipW=recipW, cnt=cnt)


def weight_inputs(W_pos, b_pos, W0, b0, W1, b1, Ws, Wc, b_c):
    W_pos, W0, W1, Ws, Wc = [np.ascontiguousarray(x, F32)
                             for x in (W_pos, W0, W1, Ws, Wc)]
    wpos4 = np.zeros((4, D2), F32)
    wpos4[:3] = W_pos
    bias = np.zeros((128, NBIAS), F32)
    bias[:, 0] = np.asarray(b_pos, F32)[:128]
    bias[:, 1] = np.asarray(b_pos, F32)[128:]
    bias[:, 2:7] = np.asarray(b0, F32).T
    bias[:, 7:12] = np.asarray(b1, F32).T
    bias[:, 12] = np.asarray(b_c, F32)
    iota4 = np.zeros((128, 4), F32)
    for j in range(4):
        iota4[:, j] = np.arange(128) + 128 * j
    iota_bc = np.broadcast_to(np.arange(WIN, dtype=F32), (128, WIN)).copy()
    ident = np.eye(128, dtype=F32)
    ones1 = np.ones((128, 1), F32)
    return dict(wpos4=wpos4, w0=W0, w1=W1, ws=Ws, wc=Wc, bias=bias,
                iota4=iota4, iota_bc=iota_bc, ident=ident, ones1=ones1)


# ================================================================ bass build
def build_bass():
    if "/opt/trn_rl_repo" not in sys.path:
        sys.path.insert(0, "/opt/trn_rl_repo")
    import concourse.bass as bass
    import concourse.mybir as mybir
    from concourse import bacc, tile
    from contextlib import ExitStack

    dt = mybir.dt.float32
    dh = mybir.dt.float16
    di = mybir.dt.int32
    AF = mybir.ActivationFunctionType
    OP = mybir.AluOpType
    GELU = AF.Gelu_apprx_tanh
    EV = mybir.EngineType

    nc = bacc.Bacc("TRN2")
    # -------- dram io
    d_pts = nc.dram_tensor("pts_flat", [128, FW], dt, kind="ExternalInput")
    d_lbT = nc.dram_tensor("lbT", [128, NSTRIP], dt, kind="ExternalInput")
    d_lbTi = nc.dram_tensor("lbTi", [128, NSTRIP], di, kind="ExternalInput")
    d_gbr = nc.dram_tensor("gbr", [NPAIR, 1024], dh, kind="ExternalInput")
    d_wb = nc.dram_tensor("wbase", [1, 32], di, kind="ExternalInput")
    d_rcw = nc.dram_tensor("recipW", [128, 4 * NPAIR], dt, kind="ExternalInput")
    d_wpos4 = nc.dram_tensor("wpos4", [4, D2], dt, kind="ExternalInput")
    d_w0 = nc.dram_tensor("w0", [NBLK, D2, HID], dt, kind="ExternalInput")
    d_w1 = nc.dram_tensor("w1", [NBLK, HID, HID], dt, kind="ExternalInput")
    d_ws = nc.dram_tensor("ws", [NBLK, D2, HID], dt, kind="ExternalInput")
    d_wc = nc.dram_tensor("wc", [HID, HID], dt, kind="ExternalInput")
    d_bias = nc.dram_tensor("bias", [128, NBIAS], dt, kind="ExternalInput")
    d_iota4 = nc.dram_tensor("iota4", [128, 4], dt, kind="ExternalInput")
    d_iotab = nc.dram_tensor("iota_bc", [128, WIN], dt, kind="ExternalInput")
    d_ident = nc.dram_tensor("ident", [128, 128], dt, kind="ExternalInput")
    d_ones1 = nc.dram_tensor("ones1", [128, 1], dt, kind="ExternalInput")
    d_out = nc.dram_tensor("out_grid", [128, NBINS], dt, kind="ExternalOutput")
    d_scr = nc.dram_tensor("pt_scratch", [4, NPTS], dt)   # internal scratch

    with tile.TileContext(nc) as tc, ExitStack() as ctx:
        cpool = ctx.enter_context(tc.tile_pool(name="const", bufs=1))
        spool = ctx.enter_context(tc.tile_pool(name="stage", bufs=2))
        psumT = ctx.enter_context(tc.tile_pool(name="psumT", bufs=2, space="PSUM"))
        psumS = ctx.enter_context(tc.tile_pool(name="psumS", bufs=1, space="PSUM"))
        psumP = ctx.enter_context(tc.tile_pool(name="psumP", bufs=1, space="PSUM"))
        psumM = ctx.enter_context(tc.tile_pool(name="psumM", bufs=2, space="PSUM"))

        breg = {ev: nc.alloc_registers(f"wbase_{ev.name}", engines=(ev,))
                for ev in (EV.DVE, EV.Activation)}

        def load_base(P, ev):
            nc.engines[ev].reg_load(breg[ev], wb[0:1, P:P + 1])
            return nc.snap(breg[ev], donate=True, min_val=0,
                           max_val=NBINS - WIN)

        # ---------------- persistent sbuf
        net = cpool.tile([128, NPTS], dt, tag="net")
        sums = cpool.tile([128, NBINS], dt, tag="sums")
        lbT = cpool.tile([128, NSTRIP], dt, tag="lbT")
        lbTi = cpool.tile([128, NSTRIP], di, tag="lbTi")
        rcw = cpool.tile([128, 4 * NPAIR], dt, tag="rcw")
        wb = cpool.tile([1, 32], di, tag="wb")
        bias = cpool.tile([128, NBIAS], dt, tag="bias")
        iota4 = cpool.tile([128, 4], dt, tag="iota4")
        iotab = cpool.tile([128, WIN], dt, tag="iotab")
        ident = cpool.tile([128, 128], dt, tag="ident")
        ones1 = cpool.tile([128, 1], dt, tag="ones1")
        wpos = cpool.tile([4, D2], dt, tag="wpos")
        w0a = [cpool.tile([128, HID], dt, tag=f"w0a{i}", name=f"w0a{i}") for i in range(NBLK)]
        w0b = [cpool.tile([128, HID], dt, tag=f"w0b{i}", name=f"w0b{i}") for i in range(NBLK)]
        w1 = [cpool.tile([128, HID], dt, tag=f"w1{i}", name=f"w1{i}") for i in range(NBLK)]
        wsa = [cpool.tile([128, HID], dt, tag=f"wsa{i}", name=f"wsa{i}") for i in range(NBLK)]
        wsb = [cpool.tile([128, HID], dt, tag=f"wsb{i}", name=f"wsb{i}") for i in range(NBLK)]
        wc = cpool.tile([128, HID], dt, tag="wc")

        nc.sync.dma_start(lbT[:], d_lbT[:])
        nc.sync.dma_start(lbTi[:], d_lbTi[:])
        nc.sync.dma_start(rcw[:], d_rcw[:])
        nc.sync.dma_start(wb[:], d_wb[:])
        nc.sync.dma_start(bias[:], d_bias[:])
        nc.sync.dma_start(iota4[:], d_iota4[:])
        nc.sync.dma_start(iotab[:], d_iotab[:])
        nc.sync.dma_start(ident[:], d_ident[:])
        nc.sync.dma_start(ones1[:], d_ones1[:])
        nc.sync.dma_start(wpos[:], d_wpos4[:])
        for i in range(NBLK):
            nc.sync.dma_start(w0a[i][:], d_w0[i, 0:128, :])
            nc.sync.dma_start(w0b[i][:], d_w0[i, 128:256, :])
            nc.sync.dma_start(w1[i][:], d_w1[i, :, :])
            nc.sync.dma_start(wsa[i][:], d_ws[i, 0:128, :])
            nc.sync.dma_start(wsb[i][:], d_ws[i, 128:256, :])
        nc.sync.dma_start(wc[:], d_wc[:])

        # ---------------- pt = 2*frac(clip(p+.5)*res) - 1, flat layout
        pflat = spool.tile([128, FW], dt, tag="pre", bufs=1, name="pflat")
        nc.sync.dma_start(pflat[:], d_pts[:])
        nc.vector.tensor_scalar(pflat[:], pflat[:], 0.5, 1.0 - 1e-6, OP.add, OP.min)
        nc.vector.tensor_scalar(pflat[:], pflat[:], 1e-6, float(RES), OP.max, OP.mult)
        ci = spool.tile([128, FW], di, tag="pre2", bufs=1, name="ci")
        nc.vector.tensor_copy(ci[:], pflat[:])
        nc.vector.tensor_tensor(pflat[:], pflat[:], ci[:], OP.subtract)
        # frac<0 fix: pflat += (pflat < 0)
        nc.vector.scalar_tensor_tensor(pflat[:], pflat[:], 0.0, pflat[:],
                                       OP.is_lt, OP.add)
        nc.vector.tensor_scalar(pflat[:], pflat[:], 2.0, -1.0, OP.mult, OP.add)
        scr_flat = d_scr[:].rearrange("a (b f) -> (a b) f", f=FW)
        nc.sync.dma_start(scr_flat, pflat[:])

        def evac(dst, src, bias_col=None, gelu=False, eng="act"):
            if eng == "act":
                f = GELU if gelu else (
                    AF.Identity if bias_col is not None else AF.Copy)
                nc.scalar.activation(
                    dst, src, f,
                    bias=bias_col if bias_col is not None else 0.0)
            else:
                assert not gelu
                if bias_col is not None:
                    nc.vector.tensor_scalar(dst, src, bias_col, None, OP.add)
                else:
                    nc.vector.tensor_copy(dst, src)

        # ---------------- setup: pos-mlp + resblock 0, per 512-chunk
        for c in range(NCHUNK):
            ptc = spool.tile([4, 512], dt, tag="ptc", name="ptc")
            nc.sync.dma_start(ptc[:], d_scr[:, c * 512:(c + 1) * 512])
            x0a = psumM.tile([128, 512], dt, tag="mm")
            x0b = psumS.tile([128, 512], dt, tag="spA", name="x0b")
            nc.tensor.matmul(x0a[:], wpos[:, 0:128], ptc[:], start=True, stop=True)
            nc.tensor.matmul(x0b[:], wpos[:, 128:256], ptc[:], start=True, stop=True)
            gxa = spool.tile([128, 512], dt, tag="sgxa", bufs=1, name="gxa")
            gxb = spool.tile([128, 512], dt, tag="sgxb", bufs=1, name="gxb")
            rxa = spool.tile([128, 512], dt, tag="srxa", bufs=1, name="rxa")
            rxb = spool.tile([128, 512], dt, tag="srxb", bufs=1, name="rxb")
            evac(gxa[:], x0a[:], bias[:, 0:1], gelu=True)
            evac(gxb[:], x0b[:], bias[:, 1:2], gelu=True)
            evac(rxa[:], x0a[:], bias[:, 0:1], eng="dve")
            evac(rxb[:], x0b[:], bias[:, 1:2], eng="dve")
            hp = psumM.tile([128, 512], dt, tag="mm", name="hp0")
            nc.tensor.matmul(hp[:], w0a[0][:], gxa[:], start=True, stop=False)
            nc.tensor.matmul(hp[:], w0b[0][:], gxb[:], start=False, stop=True)
            gh = spool.tile([128, 512], dt, tag="ghs", name="gh0")
            evac(gh[:], hp[:], bias[:, 2:3], gelu=True)
            npp = psumM.tile([128, 512], dt, tag="mm", name="npp0")
            nc.tensor.matmul(npp[:], wsa[0][:], rxa[:], start=True, stop=False)
            nc.tensor.matmul(npp[:], wsb[0][:], rxb[:], start=False, stop=False)
            nc.tensor.matmul(npp[:], w1[0][:], gh[:], start=False, stop=True)
            evac(net[:, c * 512:(c + 1) * 512], npp[:], bias[:, 7:8], eng="dve")

        # ---------------- scatter: one-hot matmuls into per-pair bin windows
        def make_oh(oh, s, half):
            """one-hot [128 pts, 2*WIN] for strips s, s+1 (strip-local bins)."""
            for k in range(2):
                t = s + k
                dst = oh[:, k * WIN:(k + 1) * WIN]
                eng = nc.gpsimd if k == 1 else nc.vector
                eng.tensor_scalar(dst, iotab[:], lbT[:, t:t + 1], None,
                                  OP.is_equal)

        def scatter_pass(src_of_chunk):
            nc.vector.memset(sums[:, 0:NBINS // 2], 0.0)
            nc.gpsimd.memset(sums[:, NBINS // 2:], 0.0)
            for P in range(NPAIR):
                sps = []
                for h in range(2):
                    src = src_of_chunk(2 * P + h)
                    tp = psumT.tile([128, 512], dt, tag="tp", name="tp")
                    for t in range(4):
                        nc.tensor.transpose(tp[:, t * 128:(t + 1) * 128],
                                            src[:, t * 128:(t + 1) * 128],
                                            ident[:])
                    ntT = spool.tile([128, 512], dt, tag="ntT", name="ntT")
                    nc.vector.tensor_copy(ntT[:], tp[:])
                    oh0 = spool.tile([128, 2 * WIN], dt, tag="oh", bufs=1, name="oh0")
                    oh1 = spool.tile([128, 2 * WIN], dt, tag="oh2", bufs=1, name="oh1")
                    make_oh(oh0, 8 * P + 4 * h, h)
                    make_oh(oh1, 8 * P + 4 * h + 2, h)
                    sp = psumS.tile([128, WIN], dt, tag=("spA", "spB")[h],
                                    name="sp")
                    for t in range(4):
                        ohx = (oh0, oh1)[t // 2]
                        nc.tensor.matmul(sp[:], ntT[:, t * 128:(t + 1) * 128],
                                         ohx[:, (t % 2) * WIN:(t % 2 + 1) * WIN],
                                         start=(t == 0), stop=(t == 3))
                    sps.append(sp)
                base = load_base(P, EV.DVE)
                dst = sums[:, bass.ds(base, WIN)]
                nc.vector.tensor_tensor(dst, dst, sps[0][:], OP.add)
                nc.vector.tensor_tensor(dst, dst, sps[1][:], OP.add)

        # ---------------- pooling iterations
        for i in range(1, NBLK):
            scatter_pass(lambda c: net[:, c * 512:(c + 1) * 512])
            for P in range(NPAIR):
                baseA = load_base(P, EV.Activation)
                wstage = spool.tile([128, WIN], dt, tag="wstage")
                nc.scalar.activation(wstage[:], sums[:, bass.ds(baseA, WIN)],
                                     AF.Copy)
                mtp = psumT.tile([128, 512], dt, tag="tp", name="mtp")
                for bl in range(4):
                    nc.tensor.transpose(mtp[:, bl * 128:(bl + 1) * 128],
                                        wstage[:, bl * 128:(bl + 1) * 128],
                                        ident[:])
                mT = spool.tile([128, WIN], dt, tag="mT", name="mT")
                for bl in range(4):
                    nc.vector.tensor_scalar(mT[:, bl * 128:(bl + 1) * 128],
                                            mtp[:, bl * 128:(bl + 1) * 128],
                                            rcw[:, 4 * P + bl:4 * P + bl + 1],
                                            None, OP.mult)
                gbbm = spool.tile([128, 1024], dh, tag="gbbm", bufs=1)
                nc.sync.dma_start(gbbm[:], d_gbr[P:P + 1, :].to_broadcast((128, 1024)))
                pl = psumP.tile([128, 1024], dt, tag="pl")
                for bl in range(4):
                    ohg = spool.tile([128, 1024], dt, tag=("ohg", "ohg2")[bl % 2],
                                     bufs=1, name="ohg")
                    eng = nc.gpsimd if bl >= 2 else nc.vector
                    eng.tensor_scalar(ohg[:], gbbm[:], iota4[:, bl:bl + 1], None,
                                      OP.is_equal)
                    for h in range(2):
                        nc.tensor.matmul(pl[:, h * 512:(h + 1) * 512],
                                         mT[:, bl * 128:(bl + 1) * 128],
                                         ohg[:, h * 512:(h + 1) * 512],
                                         start=(bl == 0), stop=(bl == 3))
                gpool = spool.tile([128, 1024], dt, tag="gpool", bufs=1)
                rpool = spool.tile([128, 1024], dt, tag="rpool", bufs=1)
                evac(gpool[:], pl[:], gelu=True)
                evac(rpool[:], pl[:], eng="dve")
                ncur = net[:, P * 1024:(P + 1) * 1024]
                gnet = spool.tile([128, 1024], dt, tag="gnet", bufs=1)
                evac(gnet[:], ncur, gelu=True)
                for h in range(2):
                    cs = slice(h * 512, (h + 1) * 512)
                    ns = slice(P * 1024 + h * 512, P * 1024 + (h + 1) * 512)
                    hp = psumM.tile([128, 512], dt, tag="mm", name="hpi")
                    nc.tensor.matmul(hp[:], w0a[i][:], gnet[:, cs], start=True, stop=False)
                    nc.tensor.matmul(hp[:], w0b[i][:], gpool[:, cs], start=False, stop=True)
                    npp = psumM.tile([128, 512], dt, tag="mm", name="nppi")
                    nc.tensor.matmul(npp[:], wsa[i][:], net[:, ns], start=True, stop=False)
                    nc.tensor.matmul(npp[:], wsb[i][:], rpool[:, cs], start=False, stop=False)
                    gh = spool.tile([128, 512], dt, tag="ghs", name="ghi")
                    evac(gh[:], hp[:], bias[:, 2 + i:3 + i], gelu=True)
                    nc.tensor.matmul(npp[:], w1[i][:], gh[:], start=False, stop=True)
                    evac(net[:, ns], npp[:], bias[:, 7 + i:8 + i], eng="dve")

        # ---------------- head: c = net @ Wc + b_c, scatter, out (raw sums)
        def head_src(c):
            cp = psumM.tile([128, 512], dt, tag="mm", name="cp")
            nc.tensor.matmul(cp[:], wc[:], net[:, c * 512:(c + 1) * 512],
                             start=True, stop=True)
            cv = spool.tile([128, 512], dt, tag="wstage", name="cv")
            evac(cv[:], cp[:], bias[:, 12:13], eng="act")
            return cv[:]

        scatter_pass(head_src)
        nc.sync.dma_start(d_out[:], sums[:])

    return nc


# ================================================================ run + glue
_BUILT = {}


def get_nc():
    if "nc" not in _BUILT:
        nc = build_bass()
        nc.compile()          # bacc pipeline: reg alloc, library loads, ...
        _BUILT["nc"] = nc
    return _BUILT["nc"]


def make_in_maps(p, sparse_coords, W_pos, b_pos, W0, b0, W1, b1, Ws, Wc, b_c, res):
    index, counts = point_meta(p, sparse_coords, int(res))
    shards = shard(np.asarray(p, F32), index)
    wdict = weight_inputs(W_pos, b_pos, W0, b0, W1, b1, Ws, Wc, b_c)
    in_maps = []
    for sh in shards:
        ci = core_inputs(sh)
        sh["cnt"] = ci["cnt"]
        m = dict(pts_flat=ci["pts_flat"], lbT=ci["lbT"], lbTi=ci["lbTi"],
                 gbr=ci["gbr"], wbase=ci["wbase"], recipW=ci["recipW"],
                 wpos4=wdict["wpos4"], w0=wdict["w0"], w1=wdict["w1"],
                 ws=wdict["ws"], wc=wdict["wc"], bias=wdict["bias"],
                 iota4=wdict["iota4"], iota_bc=wdict["iota_bc"],
                 ident=wdict["ident"], ones1=wdict["ones1"])
        in_maps.append(m)
    return in_maps, shards, counts


def assemble(results, shards, counts, sparse_coords):
    sc = np.asarray(sparse_coords)
    starts = np.concatenate([[0], np.cumsum(counts)[:-1]])
    out = np.zeros((sc.shape[0], HID), F32)
    for sh, r_ in zip(shards, results):
        tab = np.asarray(r_["out_grid"])              # [128, NBINS] raw sums
        lo, hi, b = sh["lo"], sh["hi"], sh["batch"]
        hi_eff = min(hi, int(counts[b]))
        if hi_eff > lo:
            k = hi_eff - lo
            denom = np.maximum(sh["cnt"][:k], F32(1.0))
            out[starts[b] + lo: starts[b] + hi_eff] = (tab[:, 0:k] / denom).T
    return out


def kernel(p, sparse_coords, W_pos, b_pos, W0, b0, W1, b1, Ws, Wc, b_c, res):
    if "/opt/trn_rl_repo" not in sys.path:
        sys.path.insert(0, "/opt/trn_rl_repo")
    from concourse.bass_utils import run_bass_kernel_spmd

    in_maps, shards, counts = make_in_maps(
        p, sparse_coords, W_pos, b_pos, W0, b0, W1, b1, Ws, Wc, b_c, res)
    nc = get_nc()
    results = run_bass_kernel_spmd(nc, in_maps, list(range(NCORES))).results
    return assemble(results, shards, counts, sparse_coords)


# revision 11
# speedup vs baseline: 3.5179x; 3.5179x over previous
"""LocalPoolPointnet on 8 Trainium2 cores (v3).

Points sorted by (batch, sparse-bin) on the host; each core owns a contiguous
bin range of one batch plus all points in it.  scatter_mean / gather are
core-local one-hot matmuls in fp32.  v3 structural changes vs the original:

- one dynamic 512-bin window per PAIR of 512-pt chunks (max span 455 < 512):
  halves window adds, snaps, wstage copies and mean transposes.
- raw bin sums on device; per-bin 1/count folded into the mean-window
  transpose evac (recipW, host-precomputed per pair-block); the final output
  is divided by counts on the host.
- gather one-hot is a plain is_equal (no per-point recip multiply, no rp
  broadcast DMA); scatter one-hot built by gpsimd local_scatter (or DVE
  is_equal fallback), freeing the vector engine.
- gather matmuls and resblock evacs run on 1024-pt tiles; independent psum
  banks are interleaved (scatter uses two alternating accumulators) so fp32
  matmul passes overlap.
"""

import sys
import numpy as np

# ---------------------------------------------------------------- constants
B = 2
NP_ = 100_000
HID = 128
D2 = 256
NBLK = 5
RES = 64
R = 20_005          # max_coord_num in the reference
BIG = RES ** 3 + 1
NCORES = 8
CORES_PER_BATCH = NCORES // B

NPTS = 25_600       # padded points per core  (= 50 * 512)
NCHUNK = NPTS // 512            # 50
NPAIR = NCHUNK // 2             # 25
NSTRIP = NPTS // 128            # 200
WIN = 512                       # bin window per 1024-pt pair
NBINS = 6_016                   # padded bins per core (= 47 * 128)
NBIAS = 13                      # b_pos(2) b0(5) b1(5) b_c(1)
FW = NPTS * 4 // 128            # 784

USE_LOCAL_SCATTER = False       # local_scatter needs 2-byte out: unusable

F32 = np.float32
F16 = np.float16


# ================================================================ host prep
def point_meta(p, sparse_coords, res):
    """Integer routing metadata, bit-identical to the reference's indexing."""
    p = np.asarray(p, F32)
    sc = np.asarray(sparse_coords)
    coord = np.clip(p + F32(0.5), F32(1e-6), F32(1.0 - 1e-6)) * F32(res)
    cl = coord.astype(np.int32)
    lin = (cl[..., 0] * res + cl[..., 1]) * res + cl[..., 2]      # [B, NP]

    slin = (sc[:, 1] * res + sc[:, 2]) * res + sc[:, 3]
    index = np.empty((B, NP_), np.int64)
    for b in range(B):
        coords_b = np.sort(np.where(sc[:, 0] == b, slin, BIG))
        index[b] = np.searchsorted(coords_b, lin[b], side="left")
    counts = np.bincount(sc[:, 0], minlength=B)
    return index, counts


def shard(p, index):
    """Split each batch's points into CORES_PER_BATCH contiguous-bin shards."""
    shards = []
    for b in range(B):
        idx = index[b]
        order = np.argsort(idx, kind="stable")
        sidx = idx[order]
        binc = np.bincount(idx, minlength=R)
        csum = np.cumsum(binc)
        prev_hi = 0
        for c in range(CORES_PER_BATCH):
            if c < CORES_PER_BATCH - 1:
                target = (c + 1) * NP_ // CORES_PER_BATCH
                hi = int(np.searchsorted(csum, target))
                if hi > 0 and target - csum[hi - 1] < csum[hi] - target:
                    hi -= 1
                hi += 1          # shard owns bins [lo, hi)
            else:
                hi = R
            lo = prev_hi
            prev_hi = hi
            sel = slice(int(np.searchsorted(sidx, lo)), int(np.searchsorted(sidx, hi)))
            pts = p[b][order[sel]]                     # [n, 3] sorted by bin
            rel = (sidx[sel] - lo).astype(np.int64)    # sorted rel bins
            assert pts.shape[0] <= NPTS, f"core shard too big: {pts.shape[0]}"
            nb = hi - lo
            assert nb <= NBINS, f"bin shard too big: {nb}"
            shards.append(dict(batch=b, lo=lo, hi=hi, pts=pts, rel=rel, nb=nb))
    return shards


def core_inputs(sh):
    """Per-core padded arrays for the device kernel."""
    n = sh["pts"].shape[0]
    pts = np.full((NPTS, 3), 0.25, F32)
    pts[:n] = sh["pts"]
    rel = sh["rel"]

    lb = np.full(NPTS, -1.0, F32)       # bin - pair window base (-1 dummies)
    wbase = np.zeros(NPAIR, np.int32)   # window base per 1024-pt pair
    for P in range(NPAIR):
        s, e = P * 1024, min((P + 1) * 1024, n)
        if s >= n:
            break
        base = min(int(rel[s]), NBINS - WIN)
        span = int(rel[e - 1]) - base + 1
        assert span <= WIN, f"window overflow: span={span}"
        wbase[P] = base
        lb[s:e] = (rel[s:e] - base).astype(F32)

    cnt = np.bincount(rel, minlength=NBINS).astype(F32)
    recip = F32(1.0) / np.maximum(cnt, F32(1.0))
    recip_pad = np.concatenate([recip, np.ones(WIN, F32)])
    recipW = np.zeros((128, 4 * NPAIR), F32)
    for P in range(NPAIR):
        for bl in range(4):
            recipW[:, 4 * P + bl] = recip_pad[wbase[P] + 128 * bl:
                                              wbase[P] + 128 * bl + 128]

    pts4 = np.zeros((4, NPTS), F32)
    pts4[:3] = pts.T
    pts_flat = np.ascontiguousarray(pts4).reshape(128, FW)
    lbT = np.ascontiguousarray(lb.reshape(NSTRIP, 128).T)          # [128, NSTRIP]
    lbTi = lbT.astype(np.int32)
    gbr = lb.reshape(NPAIR, 1024).astype(F16)
    wb = np.zeros((1, 32), np.int32)
    wb[0, :NPAIR] = wbase
    return dict(pts_flat=pts_flat, lbT=lbT, lbTi=lbTi, gbr=gbr, wbase=wb,
                recipW=recipW, cnt=cnt)


def weight_inputs(W_pos, b_pos, W0, b0, W1, b1, Ws, Wc, b_c):
    W_pos, W0, W1, Ws, Wc = [np.ascontiguousarray(x, F32)
                             for x in (W_pos, W0, W1, Ws, Wc)]
    wpos4 = np.zeros((4, D2), F32)
    wpos4[:3] = W_pos
    bias = np.zeros((128, NBIAS), F32)
    bias[:, 0] = np.asarray(b_pos, F32)[:128]
    bias[:, 1] = np.asarray(b_pos, F32)[128:]
    bias[:, 2:7] = np.asarray(b0, F32).T
    bias[:, 7:12] = np.asarray(b1, F32).T
    bias[:, 12] = np.asarray(b_c, F32)
    iota4 = np.zeros((128, 4), F32)
    for j in range(4):
        iota4[:, j] = np.arange(128) + 128 * j
    iota_bc = np.broadcast_to(np.arange(WIN, dtype=F32), (128, WIN)).copy()
    ident = np.eye(128, dtype=F32)
    ones1 = np.ones((128, 1), F32)
    return dict(wpos4=wpos4, w0=W0, w1=W1, ws=Ws, wc=Wc, bias=bias,
                iota4=iota4, iota_bc=iota_bc, ident=ident, ones1=ones1)


# ================================================================ bass build
def build_bass():
    if "/opt/trn_rl_repo" not in sys.path:
        sys.path.insert(0, "/opt/trn_rl_repo")
    import concourse.bass as bass
    import concourse.mybir as mybir
    from concourse import bacc, tile
    from contextlib import ExitStack

    dt = mybir.dt.float32
    dh = mybir.dt.float16
    di = mybir.dt.int32
    AF = mybir.ActivationFunctionType
    OP = mybir.AluOpType
    GELU = AF.Gelu_apprx_tanh
    EV = mybir.EngineType

    nc = bacc.Bacc("TRN2")
    # -------- dram io
    d_pts = nc.dram_tensor("pts_flat", [128, FW], dt, kind="ExternalInput")
    d_lbT = nc.dram_tensor("lbT", [128, NSTRIP], dt, kind="ExternalInput")
    d_lbTi = nc.dram_tensor("lbTi", [128, NSTRIP], di, kind="ExternalInput")
    d_gbr = nc.dram_tensor("gbr", [NPAIR, 1024], dh, kind="ExternalInput")
    d_wb = nc.dram_tensor("wbase", [1, 32], di, kind="ExternalInput")
    d_rcw = nc.dram_tensor("recipW", [128, 4 * NPAIR], dt, kind="ExternalInput")
    d_wpos4 = nc.dram_tensor("wpos4", [4, D2], dt, kind="ExternalInput")
    d_w0 = nc.dram_tensor("w0", [NBLK, D2, HID], dt, kind="ExternalInput")
    d_w1 = nc.dram_tensor("w1", [NBLK, HID, HID], dt, kind="ExternalInput")
    d_ws = nc.dram_tensor("ws", [NBLK, D2, HID], dt, kind="ExternalInput")
    d_wc = nc.dram_tensor("wc", [HID, HID], dt, kind="ExternalInput")
    d_bias = nc.dram_tensor("bias", [128, NBIAS], dt, kind="ExternalInput")
    d_iota4 = nc.dram_tensor("iota4", [128, 4], dt, kind="ExternalInput")
    d_iotab = nc.dram_tensor("iota_bc", [128, WIN], dt, kind="ExternalInput")
    d_ident = nc.dram_tensor("ident", [128, 128], dt, kind="ExternalInput")
    d_ones1 = nc.dram_tensor("ones1", [128, 1], dt, kind="ExternalInput")
    d_out = nc.dram_tensor("out_grid", [128, NBINS], dt, kind="ExternalOutput")
    d_scr = nc.dram_tensor("pt_scratch", [4, NPTS], dt)   # internal scratch

    with tile.TileContext(nc) as tc, ExitStack() as ctx:
        cpool = ctx.enter_context(tc.tile_pool(name="const", bufs=1))
        spool = ctx.enter_context(tc.tile_pool(name="stage", bufs=2))
        psumT = ctx.enter_context(tc.tile_pool(name="psumT", bufs=2, space="PSUM"))
        psumS = ctx.enter_context(tc.tile_pool(name="psumS", bufs=1, space="PSUM"))
        psumP = ctx.enter_context(tc.tile_pool(name="psumP", bufs=1, space="PSUM"))
        psumM = ctx.enter_context(tc.tile_pool(name="psumM", bufs=2, space="PSUM"))

        breg = {ev: nc.alloc_registers(f"wbase_{ev.name}", engines=(ev,))
                for ev in (EV.DVE, EV.Activation)}

        def load_base(P, ev):
            nc.engines[ev].reg_load(breg[ev], wb[0:1, P:P + 1])
            return nc.snap(breg[ev], donate=True, min_val=0,
                           max_val=NBINS - WIN)

        # ---------------- persistent sbuf
        net = cpool.tile([128, NPTS], dt, tag="net")
        sums = cpool.tile([128, NBINS], dt, tag="sums")
        lbT = cpool.tile([128, NSTRIP], dt, tag="lbT")
        lbTi = cpool.tile([128, NSTRIP], di, tag="lbTi")
        rcw = cpool.tile([128, 4 * NPAIR], dt, tag="rcw")
        wb = cpool.tile([1, 32], di, tag="wb")
        bias = cpool.tile([128, NBIAS], dt, tag="bias")
        iota4 = cpool.tile([128, 4], dt, tag="iota4")
        iotab = cpool.tile([128, WIN], dt, tag="iotab")
        ident = cpool.tile([128, 128], dt, tag="ident")
        ones1 = cpool.tile([128, 1], dt, tag="ones1")
        wpos = cpool.tile([4, D2], dt, tag="wpos")
        w0a = [cpool.tile([128, HID], dt, tag=f"w0a{i}", name=f"w0a{i}") for i in range(NBLK)]
        w0b = [cpool.tile([128, HID], dt, tag=f"w0b{i}", name=f"w0b{i}") for i in range(NBLK)]
        w1 = [cpool.tile([128, HID], dt, tag=f"w1{i}", name=f"w1{i}") for i in range(NBLK)]
        wsa = [cpool.tile([128, HID], dt, tag=f"wsa{i}", name=f"wsa{i}") for i in range(NBLK)]
        wsb = [cpool.tile([128, HID], dt, tag=f"wsb{i}", name=f"wsb{i}") for i in range(NBLK)]
        wc = cpool.tile([128, HID], dt, tag="wc")

        nc.sync.dma_start(lbT[:], d_lbT[:])
        nc.sync.dma_start(lbTi[:], d_lbTi[:])
        nc.sync.dma_start(rcw[:], d_rcw[:])
        nc.sync.dma_start(wb[:], d_wb[:])
        nc.sync.dma_start(bias[:], d_bias[:])
        nc.sync.dma_start(iota4[:], d_iota4[:])
        nc.sync.dma_start(iotab[:], d_iotab[:])
        nc.sync.dma_start(ident[:], d_ident[:])
        nc.sync.dma_start(ones1[:], d_ones1[:])
        nc.sync.dma_start(wpos[:], d_wpos4[:])
        for i in range(NBLK):
            nc.sync.dma_start(w0a[i][:], d_w0[i, 0:128, :])
            nc.sync.dma_start(w0b[i][:], d_w0[i, 128:256, :])
            nc.sync.dma_start(w1[i][:], d_w1[i, :, :])
            nc.sync.dma_start(wsa[i][:], d_ws[i, 0:128, :])
            nc.sync.dma_start(wsb[i][:], d_ws[i, 128:256, :])
        nc.sync.dma_start(wc[:], d_wc[:])

        # ---------------- pt = 2*frac(clip(p+.5)*res) - 1, flat layout
        pflat = spool.tile([128, FW], dt, tag="pre", bufs=1, name="pflat")
        nc.sync.dma_start(pflat[:], d_pts[:])
        nc.vector.tensor_scalar(pflat[:], pflat[:], 0.5, 1.0 - 1e-6, OP.add, OP.min)
        nc.vector.tensor_scalar(pflat[:], pflat[:], 1e-6, float(RES), OP.max, OP.mult)
        ci = spool.tile([128, FW], di, tag="pre2", bufs=1, name="ci")
        nc.vector.tensor_copy(ci[:], pflat[:])
        nc.vector.tensor_tensor(pflat[:], pflat[:], ci[:], OP.subtract)
        # frac<0 fix: pflat += (pflat < 0)
        nc.vector.scalar_tensor_tensor(pflat[:], pflat[:], 0.0, pflat[:],
                                       OP.is_lt, OP.add)
        nc.vector.tensor_scalar(pflat[:], pflat[:], 2.0, -1.0, OP.mult, OP.add)
        scr_flat = d_scr[:].rearrange("a (b f) -> (a b) f", f=FW)
        nc.sync.dma_start(scr_flat, pflat[:])

        def evac(dst, src, bias_col=None, gelu=False, eng="act"):
            if eng == "act":
                f = GELU if gelu else (
                    AF.Identity if bias_col is not None else AF.Copy)
                nc.scalar.activation(
                    dst, src, f,
                    bias=bias_col if bias_col is not None else 0.0)
            else:
                assert not gelu
                if bias_col is not None:
                    nc.vector.tensor_scalar(dst, src, bias_col, None, OP.add)
                else:
                    nc.vector.tensor_copy(dst, src)

        # ---------------- setup: pos-mlp + resblock 0, per 512-chunk
        for c in range(NCHUNK):
            ptc = spool.tile([4, 512], dt, tag="ptc", name="ptc")
            nc.sync.dma_start(ptc[:], d_scr[:, c * 512:(c + 1) * 512])
            x0a = psumM.tile([128, 512], dt, tag="mm")
            x0b = psumS.tile([128, 512], dt, tag="spA", name="x0b")
            nc.tensor.matmul(x0a[:], wpos[:, 0:128], ptc[:], start=True, stop=True)
            nc.tensor.matmul(x0b[:], wpos[:, 128:256], ptc[:], start=True, stop=True)
            gxa = spool.tile([128, 512], dt, tag="sgxa", bufs=1, name="gxa")
            gxb = spool.tile([128, 512], dt, tag="sgxb", bufs=1, name="gxb")
            rxa = spool.tile([128, 512], dt, tag="srxa", bufs=1, name="rxa")
            rxb = spool.tile([128, 512], dt, tag="srxb", bufs=1, name="rxb")
            evac(gxa[:], x0a[:], bias[:, 0:1], gelu=True)
            evac(gxb[:], x0b[:], bias[:, 1:2], gelu=True)
            evac(rxa[:], x0a[:], bias[:, 0:1], eng="dve")
            evac(rxb[:], x0b[:], bias[:, 1:2], eng="dve")
            hp = psumM.tile([128, 512], dt, tag="mm", name="hp0")
            nc.tensor.matmul(hp[:], w0a[0][:], gxa[:], start=True, stop=False)
            nc.tensor.matmul(hp[:], w0b[0][:], gxb[:], start=False, stop=True)
            gh = spool.tile([128, 512], dt, tag="ghs", name="gh0")
            evac(gh[:], hp[:], bias[:, 2:3], gelu=True)
            npp = psumM.tile([128, 512], dt, tag="mm", name="npp0")
            nc.tensor.matmul(npp[:], wsa[0][:], rxa[:], start=True, stop=False)
            nc.tensor.matmul(npp[:], wsb[0][:], rxb[:], start=False, stop=False)
            nc.tensor.matmul(npp[:], w1[0][:], gh[:], start=False, stop=True)
            evac(net[:, c * 512:(c + 1) * 512], npp[:], bias[:, 7:8], eng="dve")

        # ---------------- scatter: one-hot matmuls into per-pair bin windows
        def make_oh(oh, s, half):
            """one-hot [128 pts, 2*WIN] for strips s, s+1 (strip-local bins)."""
            for k in range(2):
                t = s + k
                dst = oh[:, k * WIN:(k + 1) * WIN]
                nc.vector.tensor_scalar(dst, iotab[:], lbT[:, t:t + 1], None,
                                        OP.is_equal)

        def scatter_pass(src_of_chunk):
            nc.vector.memset(sums[:], 0.0)
            for P in range(NPAIR):
                sps = []
                for h in range(2):
                    src = src_of_chunk(2 * P + h)
                    tp = psumT.tile([128, 512], dt, tag="tp", name="tp")
                    for t in range(4):
                        nc.tensor.transpose(tp[:, t * 128:(t + 1) * 128],
                                            src[:, t * 128:(t + 1) * 128],
                                            ident[:])
                    ntT = spool.tile([128, 512], dt, tag="ntT", name="ntT")
                    nc.scalar.activation(ntT[:], tp[:], AF.Copy)
                    oh0 = spool.tile([128, 2 * WIN], dt, tag="oh", bufs=1, name="oh0")
                    oh1 = spool.tile([128, 2 * WIN], dt, tag="oh2", bufs=1, name="oh1")
                    make_oh(oh0, 8 * P + 4 * h, h)
                    make_oh(oh1, 8 * P + 4 * h + 2, h)
                    sp = psumS.tile([128, WIN], dt, tag=("spA", "spB")[h],
                                    name="sp")
                    for t in range(4):
                        ohx = (oh0, oh1)[t // 2]
                        nc.tensor.matmul(sp[:], ntT[:, t * 128:(t + 1) * 128],
                                         ohx[:, (t % 2) * WIN:(t % 2 + 1) * WIN],
                                         start=(t == 0), stop=(t == 3))
                    sps.append(sp)
                base = load_base(P, EV.DVE)
                dst = sums[:, bass.ds(base, WIN)]
                nc.vector.tensor_tensor(dst, dst, sps[0][:], OP.add)
                nc.vector.tensor_tensor(dst, dst, sps[1][:], OP.add)

        # ---------------- pooling iterations
        for i in range(1, NBLK):
            scatter_pass(lambda c: net[:, c * 512:(c + 1) * 512])
            for P in range(NPAIR):
                baseA = load_base(P, EV.Activation)
                wstage = spool.tile([128, WIN], dt, tag="wstage")
                nc.scalar.activation(wstage[:], sums[:, bass.ds(baseA, WIN)],
                                     AF.Copy)
                mtp = psumT.tile([128, 512], dt, tag="tp", name="mtp")
                for bl in range(4):
                    nc.tensor.transpose(mtp[:, bl * 128:(bl + 1) * 128],
                                        wstage[:, bl * 128:(bl + 1) * 128],
                                        ident[:])
                mT = spool.tile([128, WIN], dt, tag="mT", name="mT")
                for bl in range(4):
                    nc.scalar.activation(mT[:, bl * 128:(bl + 1) * 128],
                                         mtp[:, bl * 128:(bl + 1) * 128],
                                         AF.Copy,
                                         scale=rcw[:, 4 * P + bl:4 * P + bl + 1])
                gbbm = spool.tile([128, 1024], dh, tag="gbbm", bufs=1)
                nc.sync.dma_start(gbbm[:], d_gbr[P:P + 1, :].to_broadcast((128, 1024)))
                pl = psumP.tile([128, 1024], dt, tag="pl")
                for bl in range(4):
                    ohg = spool.tile([128, 1024], dt, tag=("ohg", "ohg2")[bl % 2],
                                     bufs=1, name="ohg")
                    nc.vector.tensor_scalar(ohg[:], gbbm[:], iota4[:, bl:bl + 1],
                                            None, OP.is_equal)
                    for h in range(2):
                        nc.tensor.matmul(pl[:, h * 512:(h + 1) * 512],
                                         mT[:, bl * 128:(bl + 1) * 128],
                                         ohg[:, h * 512:(h + 1) * 512],
                                         start=(bl == 0), stop=(bl == 3))
                gpool = spool.tile([128, 1024], dt, tag="gpool", bufs=1)
                rpool = spool.tile([128, 1024], dt, tag="rpool", bufs=1)
                evac(gpool[:], pl[:], gelu=True)
                evac(rpool[:], pl[:], eng="act")
                ncur = net[:, P * 1024:(P + 1) * 1024]
                gnet = spool.tile([128, 1024], dt, tag="gnet", bufs=1)
                evac(gnet[:], ncur, gelu=True)
                for h in range(2):
                    cs = slice(h * 512, (h + 1) * 512)
                    ns = slice(P * 1024 + h * 512, P * 1024 + (h + 1) * 512)
                    hp = psumM.tile([128, 512], dt, tag="mm", name="hpi")
                    nc.tensor.matmul(hp[:], w0a[i][:], gnet[:, cs], start=True, stop=False)
                    nc.tensor.matmul(hp[:], w0b[i][:], gpool[:, cs], start=False, stop=True)
                    npp = psumM.tile([128, 512], dt, tag="mm", name="nppi")
                    nc.tensor.matmul(npp[:], wsa[i][:], net[:, ns], start=True, stop=False)
                    nc.tensor.matmul(npp[:], wsb[i][:], rpool[:, cs], start=False, stop=False)
                    gh = spool.tile([128, 512], dt, tag="ghs", name="ghi")
                    evac(gh[:], hp[:], bias[:, 2 + i:3 + i], gelu=True)
                    nc.tensor.matmul(npp[:], w1[i][:], gh[:], start=False, stop=True)
                    evac(net[:, ns], npp[:], bias[:, 7 + i:8 + i], eng="dve")

        # ---------------- head: c = net @ Wc + b_c, scatter, out (raw sums)
        def head_src(c):
            cp = psumM.tile([128, 512], dt, tag="mm", name="cp")
            nc.tensor.matmul(cp[:], wc[:], net[:, c * 512:(c + 1) * 512],
                             start=True, stop=True)
            cv = spool.tile([128, 512], dt, tag="wstage", name="cv")
            evac(cv[:], cp[:], bias[:, 12:13], eng="act")
            return cv[:]

        scatter_pass(head_src)
        nc.sync.dma_start(d_out[:], sums[:])

    return nc


# ================================================================ run + glue
_BUILT = {}


def get_nc():
    if "nc" not in _BUILT:
        nc = build_bass()
        nc.compile()          # bacc pipeline: reg alloc, library loads, ...
        _BUILT["nc"] = nc
    return _BUILT["nc"]


def make_in_maps(p, sparse_coords, W_pos, b_pos, W0, b0, W1, b1, Ws, Wc, b_c, res):
    index, counts = point_meta(p, sparse_coords, int(res))
    shards = shard(np.asarray(p, F32), index)
    wdict = weight_inputs(W_pos, b_pos, W0, b0, W1, b1, Ws, Wc, b_c)
    in_maps = []
    for sh in shards:
        ci = core_inputs(sh)
        sh["cnt"] = ci["cnt"]
        m = dict(pts_flat=ci["pts_flat"], lbT=ci["lbT"], lbTi=ci["lbTi"],
                 gbr=ci["gbr"], wbase=ci["wbase"], recipW=ci["recipW"],
                 wpos4=wdict["wpos4"], w0=wdict["w0"], w1=wdict["w1"],
                 ws=wdict["ws"], wc=wdict["wc"], bias=wdict["bias"],
                 iota4=wdict["iota4"], iota_bc=wdict["iota_bc"],
                 ident=wdict["ident"], ones1=wdict["ones1"])
        in_maps.append(m)
    return in_maps, shards, counts


def assemble(results, shards, counts, sparse_coords):
    sc = np.asarray(sparse_coords)
    starts = np.concatenate([[0], np.cumsum(counts)[:-1]])
    out = np.zeros((sc.shape[0], HID), F32)
    for sh, r_ in zip(shards, results):
        tab = np.asarray(r_["out_grid"])              # [128, NBINS] raw sums
        lo, hi, b = sh["lo"], sh["hi"], sh["batch"]
        hi_eff = min(hi, int(counts[b]))
        if hi_eff > lo:
            k = hi_eff - lo
            denom = np.maximum(sh["cnt"][:k], F32(1.0))
            out[starts[b] + lo: starts[b] + hi_eff] = (tab[:, 0:k] / denom).T
    return out


def kernel(p, sparse_coords, W_pos, b_pos, W0, b0, W1, b1, Ws, Wc, b_c, res):
    if "/opt/trn_rl_repo" not in sys.path:
        sys.path.insert(0, "/opt/trn_rl_repo")
    from concourse.bass_utils import run_bass_kernel_spmd

    in_maps, shards, counts = make_in_maps(
        p, sparse_coords, W_pos, b_pos, W0, b0, W1, b1, Ws, Wc, b_c, res)
    nc = get_nc()
    results = run_bass_kernel_spmd(nc, in_maps, list(range(NCORES))).results
    return assemble(results, shards, counts, sparse_coords)


# revision 17
# speedup vs baseline: 7.4905x; 2.1293x over previous
"""LocalPoolPointnet on 8 Trainium2 cores (v5: segmented scans, no one-hot).

The data has ~124 occupied bins per core with 30-800 points each (points
sorted by bin).  scatter_mean + gather become three DVE passes per tile:

  fs  = tensor_tensor_scan(m, net)        # segmented running sum (m: 0 at
                                          # segment starts, else 1)
  fs *= em_rp                             # keep mean (= sum * 1/cnt) at each
                                          # segment END, zero elsewhere
  tot = reverse-scan(m>>1, fs)            # broadcast each segment's mean
                                          # back over the segment (hold scan)

All matmuls are plain fp32 resblock GEMMs; there is no bin table, no one-hot,
no transposes, no dynamic addressing on the device.  Points are packed into
static REGIONS (whole bins per region, dummy-padded); scans chain inside a
region via static initial wiring and the masks kill any cross-region carry.
The head emits masked means in point space; the host gathers segment ends.
"""

import sys
import numpy as np

# ---------------------------------------------------------------- constants
B = 2
NP_ = 100_000
HID = 128
D2 = 256
NBLK = 5
RES = 64
R = 20_005          # max_coord_num in the reference
BIG = RES ** 3 + 1
NCORES = 8
CORES_PER_BATCH = NCORES // B

NCHUNK = 52                     # 512-pt chunks per core
NPTS = NCHUNK * 512             # 26624 padded points per core
REGIONS = [6, 6, 6, 6, 6, 6, 6, 6, 4]   # chunks per region (sum = 52)
assert sum(REGIONS) == NCHUNK
NBIAS = 13                      # b_pos(2) b0(5) b1(5) b_c(1)
FW = NPTS * 4 // 128            # 832

F32 = np.float32
F16 = np.float16


# ================================================================ host prep
def point_meta(p, sparse_coords, res):
    """Integer routing metadata, bit-identical to the reference's indexing."""
    p = np.asarray(p, F32)
    sc = np.asarray(sparse_coords)
    coord = np.clip(p + F32(0.5), F32(1e-6), F32(1.0 - 1e-6)) * F32(res)
    cl = coord.astype(np.int32)
    lin = (cl[..., 0] * res + cl[..., 1]) * res + cl[..., 2]      # [B, NP]

    slin = (sc[:, 1] * res + sc[:, 2]) * res + sc[:, 3]
    index = np.empty((B, NP_), np.int64)
    for b in range(B):
        coords_b = np.sort(np.where(sc[:, 0] == b, slin, BIG))
        index[b] = np.searchsorted(coords_b, lin[b], side="left")
    counts = np.bincount(sc[:, 0], minlength=B)
    return index, counts


def shard(p, index):
    """Split each batch's points into CORES_PER_BATCH contiguous-bin shards."""
    shards = []
    for b in range(B):
        idx = index[b]
        order = np.argsort(idx, kind="stable")
        sidx = idx[order]
        binc = np.bincount(idx, minlength=R)
        csum = np.cumsum(binc)
        prev_hi = 0
        for c in range(CORES_PER_BATCH):
            if c < CORES_PER_BATCH - 1:
                target = (c + 1) * NP_ // CORES_PER_BATCH
                hi = int(np.searchsorted(csum, target))
                if hi > 0 and target - csum[hi - 1] < csum[hi] - target:
                    hi -= 1
                hi += 1          # shard owns bins [lo, hi)
            else:
                hi = R
            lo = prev_hi
            prev_hi = hi
            sel = slice(int(np.searchsorted(sidx, lo)), int(np.searchsorted(sidx, hi)))
            pts = p[b][order[sel]]                     # [n, 3] sorted by bin
            rel = (sidx[sel] - lo).astype(np.int64)    # sorted rel bins
            shards.append(dict(batch=b, lo=lo, hi=hi, pts=pts, rel=rel,
                               nb=hi - lo))
    return shards


def core_inputs(sh):
    """Whole-bin region packing + scan masks for one core."""
    n = sh["pts"].shape[0]
    rel = sh["rel"]

    pts = np.full((NPTS, 3), 0.25, F32)
    m = np.ones(NPTS + 16, F32)          # scan carry mask (0 = segment start)
    emrp = np.zeros(NPTS, F32)           # 1/cnt at segment ends, else 0
    end_pos = {}                         # rel bin -> padded end position

    # bin run boundaries
    starts = np.flatnonzero(np.r_[True, rel[1:] != rel[:-1]])
    ends = np.r_[starts[1:], n]
    nbin = len(starts)

    region_caps = [r * 512 for r in REGIONS]
    # best-fit-decreasing bin packing into regions (bin order is free)
    order = sorted(range(nbin), key=lambda k: -(ends[k] - starts[k]))
    left = list(region_caps)
    fill = [0] * len(region_caps)
    rbase = np.cumsum([0] + region_caps[:-1])
    assign = {}
    for k in order:
        ln = int(ends[k] - starts[k])
        cands = [r for r in range(len(left)) if left[r] >= ln]
        assert cands, f"bin of {ln} pts does not fit any region"
        r = min(cands, key=lambda r_: left[r_])
        assign[k] = r
        left[r] -= ln
    for k in range(nbin):
        r = assign[k]
        s, e = starts[k], ends[k]
        ln = e - s
        pos = int(rbase[r]) + fill[r]
        pts[pos:pos + ln] = sh["pts"][s:e]
        m[pos] = 0.0
        emrp[pos + ln - 1] = F32(1.0) / F32(ln)
        end_pos[int(rel[s])] = pos + ln - 1
        fill[r] += ln
    m[0] = 0.0
    # region starts always begin a new segment (kills static scan carry)
    off = 0
    for cap in region_caps:
        m[off] = 0.0
        off += cap

    cnt = np.bincount(rel, minlength=max(sh["nb"], 1)).astype(F32)

    pts4 = np.zeros((4, NPTS), F32)
    pts4[:3] = pts.T
    pts_flat = np.ascontiguousarray(pts4).reshape(128, FW)
    return dict(pts_flat=pts_flat, m_row=m[None, :].astype(F16),
                emrp=emrp.reshape(NCHUNK, 512), cnt=cnt, end_pos=end_pos)


def weight_inputs(W_pos, b_pos, W0, b0, W1, b1, Ws, Wc, b_c):
    W_pos, W0, W1, Ws, Wc = [np.ascontiguousarray(x, F32)
                             for x in (W_pos, W0, W1, Ws, Wc)]
    wpos4 = np.zeros((4, D2), F32)
    wpos4[:3] = W_pos
    bias = np.zeros((128, NBIAS), F32)
    bias[:, 0] = np.asarray(b_pos, F32)[:128]
    bias[:, 1] = np.asarray(b_pos, F32)[128:]
    bias[:, 2:7] = np.asarray(b0, F32).T
    bias[:, 7:12] = np.asarray(b1, F32).T
    bias[:, 12] = np.asarray(b_c, F32)
    return dict(wpos4=wpos4, w0=W0, w1=W1, ws=Ws, wc=Wc, bias=bias)


# ================================================================ bass build
def build_bass():
    if "/opt/trn_rl_repo" not in sys.path:
        sys.path.insert(0, "/opt/trn_rl_repo")
    import concourse.mybir as mybir
    from concourse import bacc, tile
    from contextlib import ExitStack

    dt = mybir.dt.float32
    dh = mybir.dt.float16
    AF = mybir.ActivationFunctionType
    OP = mybir.AluOpType
    GELU = AF.Gelu_apprx_tanh

    # region layout in chunks
    rbounds = []
    off = 0
    for r in REGIONS:
        rbounds.append((off, off + r))
        off += r

    nc = bacc.Bacc("TRN2")
    # -------- dram io
    d_pts = nc.dram_tensor("pts_flat", [128, FW], dt, kind="ExternalInput")
    d_m = nc.dram_tensor("m_row", [1, NPTS + 16], dh, kind="ExternalInput")
    d_emr = nc.dram_tensor("emrp", [NCHUNK, 512], dt, kind="ExternalInput")
    d_wpos4 = nc.dram_tensor("wpos4", [4, D2], dt, kind="ExternalInput")
    d_w0 = nc.dram_tensor("w0", [NBLK, D2, HID], dt, kind="ExternalInput")
    d_w1 = nc.dram_tensor("w1", [NBLK, HID, HID], dt, kind="ExternalInput")
    d_ws = nc.dram_tensor("ws", [NBLK, D2, HID], dt, kind="ExternalInput")
    d_wc = nc.dram_tensor("wc", [HID, HID], dt, kind="ExternalInput")
    d_bias = nc.dram_tensor("bias", [128, NBIAS], dt, kind="ExternalInput")
    d_out = nc.dram_tensor("out_pts", [128, NPTS], dt, kind="ExternalOutput")
    d_scr = nc.dram_tensor("pt_scratch", [4, NPTS], dt)   # internal scratch

    with tile.TileContext(nc) as tc, ExitStack() as ctx:
        cpool = ctx.enter_context(tc.tile_pool(name="const", bufs=1))
        spool = ctx.enter_context(tc.tile_pool(name="stage", bufs=2))
        psumM = ctx.enter_context(tc.tile_pool(name="psumM", bufs=4, space="PSUM"))
        psumN = ctx.enter_context(tc.tile_pool(name="psumN", bufs=4, space="PSUM"))

        # ---------------- persistent sbuf
        net = cpool.tile([128, NPTS], dt, tag="net")
        mall = cpool.tile([128, NPTS + 16], dh, tag="mall")
        bias = cpool.tile([128, NBIAS], dt, tag="bias")
        wpos = cpool.tile([4, D2], dt, tag="wpos")
        w0a = [cpool.tile([128, HID], dt, tag=f"w0a{i}", name=f"w0a{i}") for i in range(NBLK)]
        w0b = [cpool.tile([128, HID], dt, tag=f"w0b{i}", name=f"w0b{i}") for i in range(NBLK)]
        w1 = [cpool.tile([128, HID], dt, tag=f"w1{i}", name=f"w1{i}") for i in range(NBLK)]
        wsa = [cpool.tile([128, HID], dt, tag=f"wsa{i}", name=f"wsa{i}") for i in range(NBLK)]
        wsb = [cpool.tile([128, HID], dt, tag=f"wsb{i}", name=f"wsb{i}") for i in range(NBLK)]
        wc = cpool.tile([128, HID], dt, tag="wc")

        nc.sync.dma_start(mall[:], d_m[0:1, :].to_broadcast((128, NPTS + 16)))
        nc.sync.dma_start(bias[:], d_bias[:])
        nc.sync.dma_start(wpos[:], d_wpos4[:])
        for i in range(NBLK):
            nc.sync.dma_start(w0a[i][:], d_w0[i, 0:128, :])
            nc.sync.dma_start(w0b[i][:], d_w0[i, 128:256, :])
            nc.sync.dma_start(w1[i][:], d_w1[i, :, :])
            nc.sync.dma_start(wsa[i][:], d_ws[i, 0:128, :])
            nc.sync.dma_start(wsb[i][:], d_ws[i, 128:256, :])
        nc.sync.dma_start(wc[:], d_wc[:])

        # ---------------- pt = 2*frac(clip(p+.5)*res) - 1, flat layout
        pflat = spool.tile([128, FW], dt, tag="pre", bufs=1, name="pflat")
        nc.sync.dma_start(pflat[:], d_pts[:])
        nc.vector.tensor_scalar(pflat[:], pflat[:], 0.5, 1.0 - 1e-6, OP.add, OP.min)
        nc.vector.tensor_scalar(pflat[:], pflat[:], 1e-6, float(RES), OP.max, OP.mult)
        ci = spool.tile([128, FW], mybir.dt.int16, tag="pre2", bufs=1, name="ci")
        nc.vector.tensor_copy(ci[:], pflat[:])
        nc.vector.tensor_tensor(pflat[:], pflat[:], ci[:], OP.subtract)
        nc.vector.scalar_tensor_tensor(pflat[:], pflat[:], 0.0, pflat[:],
                                       OP.is_lt, OP.add)
        nc.vector.tensor_scalar(pflat[:], pflat[:], 2.0, -1.0, OP.mult, OP.add)
        scr_flat = d_scr[:].rearrange("a (b f) -> (a b) f", f=FW)
        nc.sync.dma_start(scr_flat, pflat[:])

        def evac(dst, src, bias_col=None, gelu=False, eng="act"):
            if eng == "act":
                f = GELU if gelu else (
                    AF.Identity if bias_col is not None else AF.Copy)
                nc.scalar.activation(
                    dst, src, f,
                    bias=bias_col if bias_col is not None else 0.0)
            else:
                assert not gelu
                if bias_col is not None:
                    nc.vector.tensor_scalar(dst, src, bias_col, None, OP.add)
                else:
                    nc.vector.tensor_copy(dst, src)

        # ---------------- setup: pos-mlp + resblock 0, per 512-chunk
        for c in range(NCHUNK):
            ptc = spool.tile([4, 512], dt, tag="ptc", bufs=1, name="ptc")
            nc.sync.dma_start(ptc[:], d_scr[:, c * 512:(c + 1) * 512])
            x0a = psumM.tile([128, 512], dt, tag="mm")
            x0b = psumN.tile([128, 512], dt, tag="nn", name="x0b")
            nc.tensor.matmul(x0a[:], wpos[:, 0:128], ptc[:], start=True, stop=True)
            nc.tensor.matmul(x0b[:], wpos[:, 128:256], ptc[:], start=True, stop=True)
            gxa = spool.tile([128, 512], dt, tag="gpool", bufs=1, name="gxa")
            gxb = spool.tile([128, 512], dt, tag="gnet", bufs=1, name="gxb")
            rxa = spool.tile([128, 512], dt, tag="fs", bufs=7, name="rxa")
            rxb = spool.tile([128, 512], dt, tag="tot", bufs=3, name="rxb")
            evac(gxa[:], x0a[:], bias[:, 0:1], gelu=True)
            evac(gxb[:], x0b[:], bias[:, 1:2], gelu=True)
            evac(rxa[:], x0a[:], bias[:, 0:1], eng="dve")
            evac(rxb[:], x0b[:], bias[:, 1:2], eng="dve")
            hp = psumM.tile([128, 512], dt, tag="mm", name="hp0")
            nc.tensor.matmul(hp[:], w0a[0][:], gxa[:], start=True, stop=False)
            nc.tensor.matmul(hp[:], w0b[0][:], gxb[:], start=False, stop=True)
            npp = psumN.tile([128, 512], dt, tag="nn", name="npp0")
            nc.tensor.matmul(npp[:], wsa[0][:], rxa[:], start=True, stop=False)
            nc.tensor.matmul(npp[:], wsb[0][:], rxb[:], start=False, stop=False)
            gh = spool.tile([128, 512], dt, tag="ghs", bufs=1, name="gh0")
            evac(gh[:], hp[:], bias[:, 2:3], gelu=True)
            nc.tensor.matmul(npp[:], w1[0][:], gh[:], start=False, stop=True)
            evac(net[:, c * 512:(c + 1) * 512], npp[:], bias[:, 7:8], eng="dve")

        # ---------------- segmented mean per region: fs, *=emrp, reverse hold
        def pooled_region(r0, r1, src_of_chunk, want_tot=True):
            """Returns list of (c, tot_tile) for chunks [r0, r1)."""
            fss = {}
            for c in range(r0, r1):
                src = src_of_chunk(c)
                fs = spool.tile([128, 512], dt, tag="fs", bufs=7, name="fs")
                init = 0.0 if c == r0 else fss[c - 1][:, 511:512]
                nc.vector.tensor_tensor_scan(
                    fs[:], mall[:, c * 512:(c + 1) * 512], src, init,
                    OP.mult, OP.add)
                fss[c] = fs
            for c in range(r0, r1):
                emr = spool.tile([128, 512], dt, tag="emr", bufs=2, name="emr")
                nc.sync.dma_start(emr[:], d_emr[c:c + 1, :].to_broadcast((128, 512)))
                nc.vector.tensor_tensor(fss[c][:], fss[c][:], emr[:], OP.mult)
            if not want_tot:
                return [(c, fss[c]) for c in range(r0, r1)]
            tots = {}
            for c in range(r1 - 1, r0 - 1, -1):
                tot = spool.tile([128, 512], dt, tag="tot", bufs=3, name="tot")
                init = 0.0 if c == r1 - 1 else tots[c + 1][:, 0:1]
                # h = m shifted left by one; reversed APs give a backward scan
                h_rev = mall[:, c * 512 + 512:c * 512:-1]
                nc.vector.tensor_tensor_scan(
                    tot[:, ::-1], h_rev, fss[c][:, ::-1], init,
                    OP.mult, OP.add)
                tots[c] = tot
            return [(c, tots[c]) for c in range(r0, r1)]

        # ---------------- pooling iterations
        for i in range(1, NBLK):
            for (r0, r1) in rbounds:
                pairs = pooled_region(r0, r1, lambda c: net[:, c * 512:(c + 1) * 512])
                for c, tot in reversed(pairs):
                    ns = slice(c * 512, (c + 1) * 512)
                    gpool = spool.tile([128, 512], dt, tag="gpool", bufs=1)
                    gnet = spool.tile([128, 512], dt, tag="gnet", bufs=1)
                    evac(gpool[:], tot[:], gelu=True)
                    evac(gnet[:], net[:, ns], gelu=True)
                    hp = psumM.tile([128, 512], dt, tag="mm", name="hpi")
                    nc.tensor.matmul(hp[:], w0a[i][:], gnet[:], start=True, stop=False)
                    nc.tensor.matmul(hp[:], w0b[i][:], gpool[:], start=False, stop=True)
                    npp = psumN.tile([128, 512], dt, tag="nn", name="nppi")
                    nc.tensor.matmul(npp[:], wsa[i][:], net[:, ns], start=True, stop=False)
                    nc.tensor.matmul(npp[:], wsb[i][:], tot[:], start=False, stop=False)
                    gh = spool.tile([128, 512], dt, tag="ghs", bufs=1, name="ghi")
                    evac(gh[:], hp[:], bias[:, 2 + i:3 + i], gelu=True)
                    nc.tensor.matmul(npp[:], w1[i][:], gh[:], start=False, stop=True)
                    evac(net[:, ns], npp[:], bias[:, 7 + i:8 + i], eng="act")

        # ---------------- head: c = net @ Wc + b_c, masked segment means out
        def head_src(c):
            cp = psumM.tile([128, 512], dt, tag="mm", name="cp")
            nc.tensor.matmul(cp[:], wc[:], net[:, c * 512:(c + 1) * 512],
                             start=True, stop=True)
            cv = spool.tile([128, 512], dt, tag="gpool", bufs=1, name="cv")
            evac(cv[:], cp[:], bias[:, 12:13], eng="act")
            return cv[:]

        for (r0, r1) in rbounds:
            outs = pooled_region(r0, r1, head_src, want_tot=False)
            for c, g in outs:
                nc.sync.dma_start(d_out[:, c * 512:(c + 1) * 512], g[:])

    return nc


# ================================================================ run + glue
_BUILT = {}


def get_nc():
    if "nc" not in _BUILT:
        nc = build_bass()
        nc.compile()
        _BUILT["nc"] = nc
    return _BUILT["nc"]


def make_in_maps(p, sparse_coords, W_pos, b_pos, W0, b0, W1, b1, Ws, Wc, b_c, res):
    index, counts = point_meta(p, sparse_coords, int(res))
    shards = shard(np.asarray(p, F32), index)
    wdict = weight_inputs(W_pos, b_pos, W0, b0, W1, b1, Ws, Wc, b_c)
    in_maps = []
    for sh in shards:
        ci = core_inputs(sh)
        sh["end_pos"] = ci["end_pos"]
        m = dict(pts_flat=ci["pts_flat"], m_row=ci["m_row"], emrp=ci["emrp"],
                 wpos4=wdict["wpos4"], w0=wdict["w0"], w1=wdict["w1"],
                 ws=wdict["ws"], wc=wdict["wc"], bias=wdict["bias"])
        in_maps.append(m)
    return in_maps, shards, counts


def assemble(results, shards, counts, sparse_coords):
    sc = np.asarray(sparse_coords)
    starts = np.concatenate([[0], np.cumsum(counts)[:-1]])
    out = np.zeros((sc.shape[0], HID), F32)
    for sh, r_ in zip(shards, results):
        g = np.asarray(r_["out_pts"])                 # [128, NPTS] masked means
        lo, b = sh["lo"], sh["batch"]
        row0 = starts[b] + lo
        for rb, pos in sh["end_pos"].items():
            out[row0 + rb] = g[:, pos]
    return out


def kernel(p, sparse_coords, W_pos, b_pos, W0, b0, W1, b1, Ws, Wc, b_c, res):
    if "/opt/trn_rl_repo" not in sys.path:
        sys.path.insert(0, "/opt/trn_rl_repo")
    from concourse.bass_utils import run_bass_kernel_spmd

    in_maps, shards, counts = make_in_maps(
        p, sparse_coords, W_pos, b_pos, W0, b0, W1, b1, Ws, Wc, b_c, res)
    nc = get_nc()
    results = run_bass_kernel_spmd(nc, in_maps, list(range(NCORES))).results
    return assemble(results, shards, counts, sparse_coords)
